# revision 24
# baseline (speedup 1.0000x reference)
"""Trainium2 Bass kernel for nn_DecoderLSTMCell.

Computes, for B=16384 rows:
    gates = y @ W.T + h0 @ U.T + ctx @ C.T + b            # [B, 4H]
    i, f, o, g = split(gates, 4); i,f,o = sigmoid; g = tanh
    c = i * g + f * c0 ; h = o * tanh(c)
Returns (c, h), both [B, H] float32.

Strategy: data-parallel over the batch dim across 8 NeuronCores (2048
rows/core), weights replicated.  The host packs x = [y|h0|ctx] and
Wcat = [W|U|C] into transposed, partition-major bf16 layouts (the GEMM
runs in bf16 with fp32 PSUM accumulation either way; packing on the host
keeps the cores on the tensor-engine roofline).  Each core streams the
packed operands, runs its [2048 x 4096 x 4096] GEMM slice, and applies
the LSTM epilogue on the DVE/ACT engines directly from PSUM.

Per-core schedule (PE roofline = 4096 matmuls x 213.3 ns = 874 us):
  - 8 warm-up matmuls on a memset scratch tile run while the first HBM
    tiles stream in, so the PE's HAM clock-gate is already at 8/8 when
    the first real matmul issues.
  - The first (pass 0, e=0) block runs k-OUTER across all 8 m-tiles /
    8 PSUM banks, so the early HBM demand is spread over ~55 us of
    compute instead of 14 us (the pairwise schedule needs ~435 GB/s
    during the first k-chain, above the ~400 GB/s the two HWDGE rings
    deliver, and stalled ~8 us).
  - Remaining blocks run 2 batch passes x 8 hidden blocks (e) x 4
    row-tile pairs; each (e, m) accumulates 32 matmuls of
    [K=128]x[M=128]x[N=512] into one PSUM bank holding [i|f|o|g] x 128
    hidden units for 128 batch rows.
Measured baseline before these changes: ~911-922 us NEFF exec per core,
max rel err vs fp32 reference ~8e-3.
"""

import ml_dtypes
import numpy as np

import concourse.tile as tile
import concourse.mybir as mybir
from concourse import bacc, bass_utils

P = 128
F32 = mybir.dt.float32
BF16 = mybir.dt.bfloat16
AF = mybir.ActivationFunctionType

# Problem shapes (hardcoded; see module docstring)
B, IN, H, CTX = 16384, 1024, 1024, 2048
KD = IN + H + CTX  # 4096 contraction dim
G = 4 * H
NCORES = 8
BC = B // NCORES  # 2048 batch rows per core
PASSES = 2
CW = 256  # batch column chunk width of the packed x^T layout
N_WARM = 10  # PE warm-up matmuls before the first data-dependent one

LAST_RESULT = None  # BassKernelResults of the most recent run (for test.py)
_NC_CACHE = None  # compiled Bass module, reused across kernel() calls


def build_nc(bc=BC, h=H, kd=KD, passes=PASSES, cw=None, wtb_bufs=3):
    """Build the per-core SPMD Bass module.

    NEFF inputs (host-packed layouts):
      xTh : [bc//cw, P, kd//P, cw] bf16, xTh[ch,p,kt,b] = x[ch*cw+b, kt*P+p]
      wTh : [h//P, P, kd//P, 4P] bf16, wTh[e,p,kt,j*P+u] = Wcat[j*h+e*P+u, kt*P+p]
      c0s : [bc, h] f32
      bb  : [P, 4h] f32, bias broadcast along partitions, grouped like wTh:
            bb[:, e*4P + j*P + u] = b[j*h + e*P + u]
    NEFF outputs: c_out, h_out [bc, h] f32.
    """
    E = h // P
    KT = kd // P
    BPP = bc // passes  # batch rows per pass
    if cw is None:
        cw = min(CW, BPP)
    NCP = BPP // cw  # x^T chunks per pass
    MT = BPP // P  # m tiles per pass
    NW = 4 * P  # psum width: [i|f|o|g] x 128 hidden cols

    nc = bacc.Bacc("TRN2", target_bir_lowering=False)
    xTh = nc.dram_tensor("xTh", (bc // cw, P, KT, cw), BF16, kind="ExternalInput")
    wTh = nc.dram_tensor("wTh", (E, P, KT, NW), BF16, kind="ExternalInput")
    c0s = nc.dram_tensor("c0s", (bc, h), F32, kind="ExternalInput")
    bb = nc.dram_tensor("bb", (P, 4 * h), F32, kind="ExternalInput")
    c_out = nc.dram_tensor("c_out", (bc, h), F32, kind="ExternalOutput")
    h_out = nc.dram_tensor("h_out", (bc, h), F32, kind="ExternalOutput")

    with (
        tile.TileContext(nc) as tc,
        tc.tile_pool(name="xp", bufs=1) as xp,
        tc.tile_pool(name="wp", bufs=wtb_bufs) as wp,
        tc.tile_pool(name="bp", bufs=2) as bp,
        tc.tile_pool(name="cp", bufs=8) as cp,
        tc.tile_pool(name="gp", bufs=3) as gp,
        tc.tile_pool(name="sp", bufs=3) as sp,
        tc.tile_pool(name="fp", bufs=1) as fp,
        tc.tile_pool(name="pp", bufs=8, space="PSUM") as pp,
    ):
        # PE warm-up: dummy matmuls on a memset scratch tile.  No HBM
        # dependency, so they run during the DMA ring spin-up and put the
        # HAM clock gate at 8/8 before the first real matmul.
        warm = bp.tile([P, 5 * P], BF16, tag="warm", name="warm")
        nc.vector.memset(warm[:], 1.0)
        # Stationary operand for the final tile's fused bias matmul: a single
        # partition row of ones, so ps += 1 * bias_row broadcast over m.
        bmm = bp.tile([P, P], BF16, tag="bmm", name="bmm")
        nc.vector.memset(bmm[:], 0.0)
        nc.vector.memset(bmm[0:1, :], 1.0)
        wps = pp.tile([P, NW], F32, tag="ps", name="warm_ps")
        for _ in range(N_WARM):
            nc.tensor.matmul(wps[:], warm[:, 0:P], warm[:, P:5 * P], start=True, stop=True)

        def epilogue(p_i, e, m, c0_t, ps, bias_t):
            row0 = p_i * BPP + m * P
            ga = gp.tile([P, NW], F32, tag="ga", name=f"ga_{p_i}_{e}_{m}")
            nc.vector.tensor_add(ga[:], ps[:], bias_t[:])
            act = gp.tile([P, NW], F32, tag="act", name=f"act_{p_i}_{e}_{m}")
            nc.scalar.activation(act[:, 0:3 * P], ga[:, 0:3 * P], AF.Sigmoid)
            nc.scalar.activation(act[:, 3 * P:4 * P], ga[:, 3 * P:4 * P], AF.Tanh)
            ct = sp.tile([P, P], F32, tag="ct", name=f"ct_{p_i}_{e}_{m}")
            nc.vector.tensor_mul(ct[:], act[:, 0:P], act[:, 3 * P:4 * P])
            fc = sp.tile([P, P], F32, tag="fc", name=f"fc_{p_i}_{e}_{m}")
            nc.vector.tensor_mul(fc[:], act[:, P:2 * P], c0_t[:])
            nc.vector.tensor_add(ct[:], ct[:], fc[:])
            nc.scalar.dma_start(
                out=c_out[row0:row0 + P, e * P:(e + 1) * P], in_=ct[:]
            )
            tct = sp.tile([P, P], F32, tag="tct", name=f"tct_{p_i}_{e}_{m}")
            nc.scalar.activation(tct[:], ct[:], AF.Tanh)
            ht = sp.tile([P, P], F32, tag="ht", name=f"ht_{p_i}_{e}_{m}")
            nc.vector.tensor_mul(ht[:], act[:, 2 * P:3 * P], tct[:])
            nc.scalar.dma_start(
                out=h_out[row0:row0 + P, e * P:(e + 1) * P], in_=ht[:]
            )

        def epilogue_split(p_i, e, m, c0_t, ps):
            """Per-gate epilogue for the final tile.  The bias was already
            accumulated into PSUM by a fused bias matmul, so ACT applies the
            activations straight from PSUM in 128-col slices and pipelines
            with the DVE muls; output DMAs ride the (idle) sync engine so
            their descriptor generation doesn't block ACT."""
            row0 = p_i * BPP + m * P
            gt = {}
            for j, fn in ((3, AF.Tanh), (0, AF.Sigmoid), (1, AF.Sigmoid), (2, AF.Sigmoid)):
                t = fp.tile([P, P], F32, tag=f"ft{j}", name=f"ft{j}")
                nc.scalar.activation(t[:], ps[:, j * P:(j + 1) * P], fn)
                gt[j] = t
            ct = fp.tile([P, P], F32, tag="fct", name="fct")
            nc.vector.tensor_mul(ct[:], gt[0][:], gt[3][:])
            fc = fp.tile([P, P], F32, tag="ffc", name="ffc")
            # f*c0 on the (idle) GpSimd engine, concurrent with i*g on DVE
            nc.gpsimd.tensor_mul(fc[:], gt[1][:], c0_t[:])
            nc.vector.tensor_add(ct[:], ct[:], fc[:])
            nc.sync.dma_start(
                out=c_out[row0:row0 + P, e * P:(e + 1) * P], in_=ct[:]
            )
            tct = fp.tile([P, P], F32, tag="ftct", name="ftct")
            nc.scalar.activation(tct[:], ct[:], AF.Tanh)
            ht = fp.tile([P, P], F32, tag="fht", name="fht")
            HP = P // 2
            nc.vector.tensor_mul(ht[:, 0:HP], gt[2][:, 0:HP], tct[:, 0:HP])
            nc.sync.dma_start(
                out=h_out[row0:row0 + P, e * P:e * P + HP], in_=ht[:, 0:HP]
            )
            nc.vector.tensor_mul(ht[:, HP:P], gt[2][:, HP:P], tct[:, HP:P])
            nc.scalar.dma_start(
                out=h_out[row0:row0 + P, e * P + HP:(e + 1) * P], in_=ht[:, HP:P]
            )

        # (k-lo, k-hi) phases of the first block: the HWDGE rings deliver
        # sub-DMAs serially (first transfer lands ~12 us, ~400 GB/s once
        # streaming), so the matmul consumption order walks the same
        # (phase, chunk) sequence the sub-DMAs were issued in, with blocks
        # big enough (16 k-tiles = 6.9 us of matmuls) to outlast the ring's
        # early per-transfer latency.
        KPHASES = ((0, 16), (16, 32))

        wt0 = None
        for p_i in range(passes):
            xtb = []
            if p_i == 0:
                # x rides the sync ring (it starts ~2 us before the scalar
                # ring — the ACT_TABLE_LOAD blocks the scalar engine's first
                # descriptor); w rides scalar.  Emission order tracks expected
                # completion order: the 8 HWDGE sem slots are handed out
                # round-robin over emission order across BOTH rings, so a DMA
                # emitted out of completion order stalls its slot chain.
                wt0 = wp.tile([P, KT, NW], BF16, tag="wtb", name="wtb_0_0")
                for mc in range(NCP):
                    xt = xp.tile([P, KT, cw], BF16, tag=f"xtb{mc}", name=f"xtb_0_{mc}")
                    xtb.append(xt)
                nc.sync.dma_start(out=xtb[0][:, 0:8], in_=xTh[0, :, 0:8])
                nc.scalar.dma_start(out=wt0[:, 0:4], in_=wTh[0, :, 0:4])
                nc.sync.dma_start(out=xtb[0][:, 8:16], in_=xTh[0, :, 8:16])
                nc.scalar.dma_start(out=wt0[:, 4:8], in_=wTh[0, :, 4:8])
                nc.sync.dma_start(out=xtb[1][:, 0:16], in_=xTh[1, :, 0:16])
                nc.scalar.dma_start(out=wt0[:, 8:16], in_=wTh[0, :, 8:16])
                nc.sync.dma_start(out=xtb[2][:, 0:16], in_=xTh[2, :, 0:16])
                nc.sync.dma_start(out=xtb[3][:, 0:16], in_=xTh[3, :, 0:16])
                nc.scalar.dma_start(out=wt0[:, 16:32], in_=wTh[0, :, 16:32])
                for mc in range(NCP):
                    nc.sync.dma_start(out=xtb[mc][:, 16:32], in_=xTh[mc, :, 16:32])
            else:
                for mc in range(NCP):
                    xt = xp.tile([P, KT, cw], BF16, tag=f"xtb{mc}", name=f"xtb_1_{mc}")
                    for q0, q1 in ((0, 8), (8, 16), (16, 24), (24, 32)):
                        nc.sync.dma_start(out=xt[:, q0:q1], in_=xTh[NCP + mc, :, q0:q1])
                    xtb.append(xt)
            for e in range(E):
                if p_i == 0 and e == 0:
                    wt = wt0  # loaded above, interleaved with the x phase slices
                else:
                    wt = wp.tile([P, KT, NW], BF16, tag="wtb", name=f"wtb_{p_i}_{e}")
                    for q0, q1 in ((0, 8), (8, 16), (16, 24), (24, 32)):
                        nc.scalar.dma_start(out=wt[:, q0:q1], in_=wTh[e, :, q0:q1])
                bias_t = bp.tile([P, NW], F32, tag="bias", name=f"bias_{p_i}_{e}")
                nc.scalar.dma_start(out=bias_t[:], in_=bb[:, e * NW:(e + 1) * NW])
                if p_i == 0 and e == 0:
                    # First block: all 8 m-tiles / 8 PSUM banks in flight, and
                    # matmuls walk (phase, chunk) in the same order the DMA
                    # sub-transfers arrive, so the PE never outruns the rings.
                    group = []
                    for m in range(MT):
                        c0_t = cp.tile([P, P], F32, tag="c0", name=f"c0_0_0_{m}")
                        nc.scalar.dma_start(
                            out=c0_t[:], in_=c0s[m * P:m * P + P, 0:P]
                        )
                        ps = pp.tile([P, NW], F32, tag="ps", name=f"ps_0_0_{m}")
                        group.append((m, c0_t, ps))
                    for q0, q1 in KPHASES:
                        for mc in range(NCP):
                            for k in range(q0, q1):
                                for m in (2 * mc, 2 * mc + 1):
                                    _, c0_t, ps = group[m]
                                    lc = (m % 2) * P
                                    nc.tensor.matmul(
                                        ps[:],
                                        xtb[mc][:, k, lc:lc + P],
                                        wt[:, k, :],
                                        start=(k == 0),
                                        stop=(k == KT - 1),
                                    )
                    for m, c0_t, ps in group:
                        epilogue(p_i, e, m, c0_t, ps, bias_t)
                    continue
                last = p_i == passes - 1 and e == E - 1
                if last:
                    # bf16 bias copy for the final tile's fused bias matmul
                    bias_bf = bp.tile([P, NW], BF16, tag="bias_bf", name="bias_bf")
                    nc.vector.tensor_copy(bias_bf[:], bias_t[:])
                pstep = 1 if last else 2
                for mp in range(0, MT, pstep):
                  pair = []
                  for m in ((mp,) if pstep == 1 else (mp, mp + 1)):
                    if m >= MT:
                        continue
                    row0 = p_i * BPP + m * P
                    c0_t = cp.tile([P, P], F32, tag="c0", name=f"c0_{p_i}_{e}_{m}")
                    ps = pp.tile([P, NW], F32, tag="ps", name=f"ps_{p_i}_{e}_{m}")
                    pair.append((m, c0_t, ps))
                  for m, c0_t, ps in pair:
                    row0 = p_i * BPP + m * P
                    nc.sync.dma_start(
                        out=c0_t[:], in_=c0s[row0:row0 + P, e * P:(e + 1) * P]
                    )
                  for k in range(KT):
                    for m, c0_t, ps in pair:
                        mc, lc = divmod(m * P, cw)
                        fuse_bias = last and m == MT - 1
                        nc.tensor.matmul(
                            ps[:],
                            xtb[mc][:, k, lc:lc + P],
                            wt[:, k, :],
                            start=(k == 0),
                            stop=(k == KT - 1 and not fuse_bias),
                        )
                  for m, c0_t, ps in pair:
                    if last and m == MT - 1:
                        nc.tensor.matmul(ps[:], bmm[:], bias_bf[:], start=False, stop=True)
                        epilogue_split(p_i, e, m, c0_t, ps)
                    else:
                        epilogue(p_i, e, m, c0_t, ps, bias_t)
    nc.compile()
    return nc


def pack_inputs(y, ctx, c0, h0, W, U, C, b, bc=BC, h=H, kd=KD, cw=CW):
    """Host-side layout packing (pure data movement, no arithmetic)."""
    b_total = y.shape[0]
    E = h // P
    KT = kd // P
    x_all = np.concatenate([y, h0, ctx], axis=1)  # [B, KD]; order matches Wcat
    xTh = np.ascontiguousarray(
        x_all.reshape(b_total // cw, cw, KT, P).transpose(0, 3, 2, 1)
    ).astype(ml_dtypes.bfloat16)
    Wcat = np.concatenate([W, U, C], axis=1)  # [G, KD]
    wTh = np.ascontiguousarray(
        Wcat.reshape(4, E, P, KT, P).transpose(1, 4, 3, 0, 2).reshape(E, P, KT, 4 * P)
    ).astype(ml_dtypes.bfloat16)
    br = b.reshape(4, E, P).transpose(1, 0, 2).reshape(4 * h)
    bb = np.ascontiguousarray(np.broadcast_to(br, (P, 4 * h)))
    return xTh, wTh, bb


def kernel(y, ctx, c0, h0, W, U, C, b):
    global LAST_RESULT
    y = np.ascontiguousarray(np.asarray(y, dtype=np.float32))
    ctx = np.ascontiguousarray(np.asarray(ctx, dtype=np.float32))
    c0 = np.ascontiguousarray(np.asarray(c0, dtype=np.float32))
    h0 = np.ascontiguousarray(np.asarray(h0, dtype=np.float32))
    W = np.ascontiguousarray(np.asarray(W, dtype=np.float32))
    U = np.ascontiguousarray(np.asarray(U, dtype=np.float32))
    C = np.ascontiguousarray(np.asarray(C, dtype=np.float32))
    b = np.ascontiguousarray(np.asarray(b, dtype=np.float32))

    xTh, wTh, bb = pack_inputs(y, ctx, c0, h0, W, U, C, b)

    global _NC_CACHE
    if _NC_CACHE is None:
        _NC_CACHE = build_nc()
    nc = _NC_CACHE
    cpb = BC // CW  # x^T chunks per core
    in_maps = []
    for c_i in range(NCORES):
        in_maps.append(
            {
                "xTh": xTh[c_i * cpb:(c_i + 1) * cpb],
                "wTh": wTh,
                "c0s": np.ascontiguousarray(c0[c_i * BC:(c_i + 1) * BC]),
                "bb": bb,
            }
        )
    res = bass_utils.run_bass_kernel_spmd(nc, in_maps, core_ids=list(range(NCORES)))
    LAST_RESULT = res
    c_full = np.concatenate([r["c_out"] for r in res.results], axis=0)
    h_full = np.concatenate([r["h_out"] for r in res.results], axis=0)
    return (c_full, h_full)


# revision 27
# speedup vs baseline: 1.0099x; 1.0099x over previous
"""Trainium2 Bass kernel for nn_DecoderLSTMCell.

Computes, for B=16384 rows:
    gates = y @ W.T + h0 @ U.T + ctx @ C.T + b            # [B, 4H]
    i, f, o, g = split(gates, 4); i,f,o = sigmoid; g = tanh
    c = i * g + f * c0 ; h = o * tanh(c)
Returns (c, h), both [B, H] float32.

Strategy: data-parallel over the batch dim across 8 NeuronCores (2048
rows/core), weights replicated.  The host packs x = [y|h0|ctx] and
Wcat = [W|U|C] into transposed, partition-major bf16 layouts (the GEMM
runs in bf16 with fp32 PSUM accumulation either way; packing on the host
keeps the cores on the tensor-engine roofline).  Each core streams the
packed operands, runs its [2048 x 4096 x 4096] GEMM slice, and applies
the LSTM epilogue on the DVE/ACT engines directly from PSUM.

Per-core schedule (PE roofline = 4096 matmuls x 213.3 ns = 874 us):
  - 8 warm-up matmuls on a memset scratch tile run while the first HBM
    tiles stream in, so the PE's HAM clock-gate is already at 8/8 when
    the first real matmul issues.
  - The first (pass 0, e=0) block runs k-OUTER across all 8 m-tiles /
    8 PSUM banks, so the early HBM demand is spread over ~55 us of
    compute instead of 14 us (the pairwise schedule needs ~435 GB/s
    during the first k-chain, above the ~400 GB/s the two HWDGE rings
    deliver, and stalled ~8 us).
  - Remaining blocks run 2 batch passes x 8 hidden blocks (e) x 4
    row-tile pairs; each (e, m) accumulates 32 matmuls of
    [K=128]x[M=128]x[N=512] into one PSUM bank holding [i|f|o|g] x 128
    hidden units for 128 batch rows.
Measured baseline before these changes: ~911-922 us NEFF exec per core,
max rel err vs fp32 reference ~8e-3.
"""

import ml_dtypes
import numpy as np

import concourse.tile as tile
import concourse.mybir as mybir
from concourse import bacc, bass_utils

P = 128
F32 = mybir.dt.float32
BF16 = mybir.dt.bfloat16
AF = mybir.ActivationFunctionType

# Problem shapes (hardcoded; see module docstring)
B, IN, H, CTX = 16384, 1024, 1024, 2048
KD = IN + H + CTX  # 4096 contraction dim
G = 4 * H
NCORES = 8
BC = B // NCORES  # 2048 batch rows per core
PASSES = 2
CW = 256  # batch column chunk width of the packed x^T layout
N_WARM = 10  # PE warm-up matmuls before the first data-dependent one

LAST_RESULT = None  # BassKernelResults of the most recent run (for test.py)
_NC_CACHE = None  # compiled Bass module, reused across kernel() calls


def build_nc(bc=BC, h=H, kd=KD, passes=PASSES, cw=None, wtb_bufs=3):
    """Build the per-core SPMD Bass module.

    NEFF inputs (host-packed layouts):
      xTh : [bc//cw, P, kd//P, cw] bf16, xTh[ch,p,kt,b] = x[ch*cw+b, kt*P+p]
      wTh : [h//P, P, kd//P, 4P] bf16, wTh[e,p,kt,j*P+u] = Wcat[j*h+e*P+u, kt*P+p]
      c0s : [bc, h] f32
      bb  : [P, 4h] f32, bias broadcast along partitions, grouped like wTh:
            bb[:, e*4P + j*P + u] = b[j*h + e*P + u]
    NEFF outputs: c_out, h_out [bc, h] f32.
    """
    E = h // P
    KT = kd // P
    BPP = bc // passes  # batch rows per pass
    if cw is None:
        cw = min(CW, BPP)
    NCP = BPP // cw  # x^T chunks per pass
    MT = BPP // P  # m tiles per pass
    NW = 4 * P  # psum width: [i|f|o|g] x 128 hidden cols

    nc = bacc.Bacc("TRN2", target_bir_lowering=False)
    xTh = nc.dram_tensor("xTh", (bc // cw, P, KT, cw), BF16, kind="ExternalInput")
    wTh = nc.dram_tensor("wTh", (E, P, KT, NW), BF16, kind="ExternalInput")
    c0s = nc.dram_tensor("c0s", (bc, h), F32, kind="ExternalInput")
    bb = nc.dram_tensor("bb", (P, 4 * h), F32, kind="ExternalInput")
    c_out = nc.dram_tensor("c_out", (bc, h), F32, kind="ExternalOutput")
    h_out = nc.dram_tensor("h_out", (bc, h), F32, kind="ExternalOutput")

    with (
        tile.TileContext(nc) as tc,
        tc.tile_pool(name="xp", bufs=1) as xp,
        tc.tile_pool(name="wp", bufs=wtb_bufs) as wp,
        tc.tile_pool(name="bp", bufs=2) as bp,
        tc.tile_pool(name="cp", bufs=8) as cp,
        tc.tile_pool(name="gp", bufs=3) as gp,
        tc.tile_pool(name="sp", bufs=3) as sp,
        tc.tile_pool(name="fp", bufs=1) as fp,
        tc.tile_pool(name="pp", bufs=8, space="PSUM") as pp,
    ):
        # PE warm-up: dummy matmuls on a memset scratch tile.  No HBM
        # dependency, so they run during the DMA ring spin-up and put the
        # HAM clock gate at 8/8 before the first real matmul.
        warm = bp.tile([P, 5 * P], BF16, tag="warm", name="warm")
        nc.vector.memset(warm[:], 1.0)
        # Stationary operand for the final tile's fused bias matmul: a single
        # partition row of ones, so ps += 1 * bias_row broadcast over m.
        bmm = bp.tile([P, P], BF16, tag="bmm", name="bmm")
        nc.vector.memset(bmm[:], 0.0)
        nc.vector.memset(bmm[0:1, :], 1.0)
        wps = pp.tile([P, NW], F32, tag="ps", name="warm_ps")
        for _ in range(N_WARM):
            nc.tensor.matmul(wps[:], warm[:, 0:P], warm[:, P:5 * P], start=True, stop=True)

        def epilogue(p_i, e, m, c0_t, ps, bias_t):
            row0 = p_i * BPP + m * P
            ga = gp.tile([P, NW], F32, tag="ga", name=f"ga_{p_i}_{e}_{m}")
            nc.vector.tensor_add(ga[:], ps[:], bias_t[:])
            act = gp.tile([P, NW], F32, tag="act", name=f"act_{p_i}_{e}_{m}")
            nc.scalar.activation(act[:, 0:3 * P], ga[:, 0:3 * P], AF.Sigmoid)
            nc.scalar.activation(act[:, 3 * P:4 * P], ga[:, 3 * P:4 * P], AF.Tanh)
            ct = sp.tile([P, P], F32, tag="ct", name=f"ct_{p_i}_{e}_{m}")
            nc.vector.tensor_mul(ct[:], act[:, 0:P], act[:, 3 * P:4 * P])
            fc = sp.tile([P, P], F32, tag="fc", name=f"fc_{p_i}_{e}_{m}")
            nc.vector.tensor_mul(fc[:], act[:, P:2 * P], c0_t[:])
            nc.vector.tensor_add(ct[:], ct[:], fc[:])
            nc.scalar.dma_start(
                out=c_out[row0:row0 + P, e * P:(e + 1) * P], in_=ct[:]
            )
            tct = sp.tile([P, P], F32, tag="tct", name=f"tct_{p_i}_{e}_{m}")
            nc.scalar.activation(tct[:], ct[:], AF.Tanh)
            ht = sp.tile([P, P], F32, tag="ht", name=f"ht_{p_i}_{e}_{m}")
            nc.vector.tensor_mul(ht[:], act[:, 2 * P:3 * P], tct[:])
            nc.scalar.dma_start(
                out=h_out[row0:row0 + P, e * P:(e + 1) * P], in_=ht[:]
            )

        def epilogue_split(p_i, e, m, c0_t, ps):
            """Per-gate epilogue for the final tile.  The bias was already
            accumulated into PSUM by a fused bias matmul, so ACT applies the
            activations straight from PSUM in 128-col slices and pipelines
            with the DVE muls; output DMAs ride the (idle) sync engine so
            their descriptor generation doesn't block ACT."""
            row0 = p_i * BPP + m * P
            gt = {}
            for j, fn in ((3, AF.Tanh), (0, AF.Sigmoid), (1, AF.Sigmoid), (2, AF.Sigmoid)):
                t = fp.tile([P, P], F32, tag=f"ft{j}", name=f"ft{j}")
                nc.scalar.activation(t[:], ps[:, j * P:(j + 1) * P], fn)
                gt[j] = t
            ct = fp.tile([P, P], F32, tag="fct", name="fct")
            nc.vector.tensor_mul(ct[:], gt[0][:], gt[3][:])
            fc = fp.tile([P, P], F32, tag="ffc", name="ffc")
            nc.vector.tensor_mul(fc[:], gt[1][:], c0_t[:])
            nc.vector.tensor_add(ct[:], ct[:], fc[:])
            nc.sync.dma_start(
                out=c_out[row0:row0 + P, e * P:(e + 1) * P], in_=ct[:]
            )
            tct = fp.tile([P, P], F32, tag="ftct", name="ftct")
            nc.scalar.activation(tct[:], ct[:], AF.Tanh)
            ht = fp.tile([P, P], F32, tag="fht", name="fht")
            nc.vector.tensor_mul(ht[:], gt[2][:], tct[:])
            nc.sync.dma_start(
                out=h_out[row0:row0 + P, e * P:(e + 1) * P], in_=ht[:]
            )

        # (k-lo, k-hi) phases of the first block.  The two HWDGE rings each
        # deliver only ~130-180 GB/s for their first handful of transfers
        # (first completion ~11-12 us), so the first block's fresh-byte
        # demand is held to ~110 GB/s per ring: chunks 0-2 ride sync,
        # chunk 3 + w ride scalar, and the matmul consumption order walks
        # the same (phase, chunk) sequence the sub-DMAs arrive in.
        KPHASES = ((0, 4), (4, 12), (12, 20), (20, 32))

        wt0 = None
        for p_i in range(passes):
            xtb = []
            if p_i == 0:
                # x rides the sync ring (it starts ~2 us before the scalar
                # ring — the ACT_TABLE_LOAD blocks the scalar engine's first
                # descriptor); w rides scalar.  Emission order tracks expected
                # completion order: the 8 HWDGE sem slots are handed out
                # round-robin over emission order across BOTH rings, so a DMA
                # emitted out of completion order stalls its slot chain.
                wt0 = wp.tile([P, KT, NW], BF16, tag="wtb", name="wtb_0_0")
                for mc in range(NCP):
                    xt = xp.tile([P, KT, cw], BF16, tag=f"xtb{mc}", name=f"xtb_0_{mc}")
                    xtb.append(xt)
                # Alternating sync/scalar emission in expected completion
                # order keeps the 8-wide global sem-slot chain stall-free.
                nc.sync.dma_start(out=xtb[0][:, 0:4], in_=xTh[0, :, 0:4])
                nc.scalar.dma_start(out=wt0[:, 0:4], in_=wTh[0, :, 0:4])
                nc.sync.dma_start(out=xtb[1][:, 0:4], in_=xTh[1, :, 0:4])
                nc.scalar.dma_start(out=xtb[3][:, 0:4], in_=xTh[3, :, 0:4])
                nc.sync.dma_start(out=xtb[2][:, 0:4], in_=xTh[2, :, 0:4])
                nc.scalar.dma_start(out=wt0[:, 4:12], in_=wTh[0, :, 4:12])
                nc.sync.dma_start(out=xtb[0][:, 4:12], in_=xTh[0, :, 4:12])
                nc.scalar.dma_start(out=xtb[3][:, 4:12], in_=xTh[3, :, 4:12])
                nc.sync.dma_start(out=xtb[1][:, 4:12], in_=xTh[1, :, 4:12])
                nc.scalar.dma_start(out=wt0[:, 12:20], in_=wTh[0, :, 12:20])
                nc.sync.dma_start(out=xtb[2][:, 4:12], in_=xTh[2, :, 4:12])
                nc.scalar.dma_start(out=xtb[3][:, 12:20], in_=xTh[3, :, 12:20])
                nc.sync.dma_start(out=xtb[0][:, 12:20], in_=xTh[0, :, 12:20])
                nc.scalar.dma_start(out=wt0[:, 20:32], in_=wTh[0, :, 20:32])
                nc.sync.dma_start(out=xtb[1][:, 12:20], in_=xTh[1, :, 12:20])
                nc.scalar.dma_start(out=xtb[3][:, 20:32], in_=xTh[3, :, 20:32])
                nc.sync.dma_start(out=xtb[2][:, 12:20], in_=xTh[2, :, 12:20])
                nc.sync.dma_start(out=xtb[0][:, 20:32], in_=xTh[0, :, 20:32])
                nc.sync.dma_start(out=xtb[1][:, 20:32], in_=xTh[1, :, 20:32])
                nc.sync.dma_start(out=xtb[2][:, 20:32], in_=xTh[2, :, 20:32])
            else:
                for mc in range(NCP):
                    xt = xp.tile([P, KT, cw], BF16, tag=f"xtb{mc}", name=f"xtb_1_{mc}")
                    for q0, q1 in ((0, 8), (8, 16), (16, 24), (24, 32)):
                        nc.sync.dma_start(out=xt[:, q0:q1], in_=xTh[NCP + mc, :, q0:q1])
                    xtb.append(xt)
            for e in range(E):
                if p_i == 0 and e == 0:
                    wt = wt0  # loaded above, interleaved with the x phase slices
                else:
                    wt = wp.tile([P, KT, NW], BF16, tag="wtb", name=f"wtb_{p_i}_{e}")
                    for q0, q1 in ((0, 8), (8, 16), (16, 24), (24, 32)):
                        nc.scalar.dma_start(out=wt[:, q0:q1], in_=wTh[e, :, q0:q1])
                bias_t = bp.tile([P, NW], F32, tag="bias", name=f"bias_{p_i}_{e}")
                nc.scalar.dma_start(out=bias_t[:], in_=bb[:, e * NW:(e + 1) * NW])
                if p_i == 0 and e == 0:
                    # First block: all 8 m-tiles / 8 PSUM banks in flight, and
                    # matmuls walk (phase, chunk) in the same order the DMA
                    # sub-transfers arrive, so the PE never outruns the rings.
                    group = []
                    for m in range(MT):
                        c0_t = cp.tile([P, P], F32, tag="c0", name=f"c0_0_0_{m}")
                        nc.scalar.dma_start(
                            out=c0_t[:], in_=c0s[m * P:m * P + P, 0:P]
                        )
                        ps = pp.tile([P, NW], F32, tag="ps", name=f"ps_0_0_{m}")
                        group.append((m, c0_t, ps))
                    for q0, q1 in KPHASES:
                        for mc in range(NCP):
                            for k in range(q0, q1):
                                for m in (2 * mc, 2 * mc + 1):
                                    _, c0_t, ps = group[m]
                                    lc = (m % 2) * P
                                    nc.tensor.matmul(
                                        ps[:],
                                        xtb[mc][:, k, lc:lc + P],
                                        wt[:, k, :],
                                        start=(k == 0),
                                        stop=(k == KT - 1),
                                    )
                    for m, c0_t, ps in group:
                        epilogue(p_i, e, m, c0_t, ps, bias_t)
                    continue
                last = p_i == passes - 1 and e == E - 1
                if last:
                    # bf16 bias copy for the final tile's fused bias matmul
                    bias_bf = bp.tile([P, NW], BF16, tag="bias_bf", name="bias_bf")
                    nc.vector.tensor_copy(bias_bf[:], bias_t[:])
                pstep = 1 if last else 2
                for mp in range(0, MT, pstep):
                  pair = []
                  for m in ((mp,) if pstep == 1 else (mp, mp + 1)):
                    if m >= MT:
                        continue
                    row0 = p_i * BPP + m * P
                    c0_t = cp.tile([P, P], F32, tag="c0", name=f"c0_{p_i}_{e}_{m}")
                    ps = pp.tile([P, NW], F32, tag="ps", name=f"ps_{p_i}_{e}_{m}")
                    pair.append((m, c0_t, ps))
                  for m, c0_t, ps in pair:
                    row0 = p_i * BPP + m * P
                    nc.sync.dma_start(
                        out=c0_t[:], in_=c0s[row0:row0 + P, e * P:(e + 1) * P]
                    )
                  for k in range(KT):
                    for m, c0_t, ps in pair:
                        mc, lc = divmod(m * P, cw)
                        fuse_bias = last and m == MT - 1
                        nc.tensor.matmul(
                            ps[:],
                            xtb[mc][:, k, lc:lc + P],
                            wt[:, k, :],
                            start=(k == 0),
                            stop=(k == KT - 1 and not fuse_bias),
                        )
                  for m, c0_t, ps in pair:
                    if last and m == MT - 1:
                        nc.tensor.matmul(ps[:], bmm[:], bias_bf[:], start=False, stop=True)
                        epilogue_split(p_i, e, m, c0_t, ps)
                    else:
                        epilogue(p_i, e, m, c0_t, ps, bias_t)
    nc.compile()
    return nc


def pack_inputs(y, ctx, c0, h0, W, U, C, b, bc=BC, h=H, kd=KD, cw=CW):
    """Host-side layout packing (pure data movement, no arithmetic)."""
    b_total = y.shape[0]
    E = h // P
    KT = kd // P
    x_all = np.concatenate([y, h0, ctx], axis=1)  # [B, KD]; order matches Wcat
    xTh = np.ascontiguousarray(
        x_all.reshape(b_total // cw, cw, KT, P).transpose(0, 3, 2, 1)
    ).astype(ml_dtypes.bfloat16)
    Wcat = np.concatenate([W, U, C], axis=1)  # [G, KD]
    wTh = np.ascontiguousarray(
        Wcat.reshape(4, E, P, KT, P).transpose(1, 4, 3, 0, 2).reshape(E, P, KT, 4 * P)
    ).astype(ml_dtypes.bfloat16)
    br = b.reshape(4, E, P).transpose(1, 0, 2).reshape(4 * h)
    bb = np.ascontiguousarray(np.broadcast_to(br, (P, 4 * h)))
    return xTh, wTh, bb


def kernel(y, ctx, c0, h0, W, U, C, b):
    global LAST_RESULT
    y = np.ascontiguousarray(np.asarray(y, dtype=np.float32))
    ctx = np.ascontiguousarray(np.asarray(ctx, dtype=np.float32))
    c0 = np.ascontiguousarray(np.asarray(c0, dtype=np.float32))
    h0 = np.ascontiguousarray(np.asarray(h0, dtype=np.float32))
    W = np.ascontiguousarray(np.asarray(W, dtype=np.float32))
    U = np.ascontiguousarray(np.asarray(U, dtype=np.float32))
    C = np.ascontiguousarray(np.asarray(C, dtype=np.float32))
    b = np.ascontiguousarray(np.asarray(b, dtype=np.float32))

    xTh, wTh, bb = pack_inputs(y, ctx, c0, h0, W, U, C, b)

    global _NC_CACHE
    if _NC_CACHE is None:
        _NC_CACHE = build_nc()
    nc = _NC_CACHE
    cpb = BC // CW  # x^T chunks per core
    in_maps = []
    for c_i in range(NCORES):
        in_maps.append(
            {
                "xTh": xTh[c_i * cpb:(c_i + 1) * cpb],
                "wTh": wTh,
                "c0s": np.ascontiguousarray(c0[c_i * BC:(c_i + 1) * BC]),
                "bb": bb,
            }
        )
    res = bass_utils.run_bass_kernel_spmd(nc, in_maps, core_ids=list(range(NCORES)))
    LAST_RESULT = res
    c_full = np.concatenate([r["c_out"] for r in res.results], axis=0)
    h_full = np.concatenate([r["h_out"] for r in res.results], axis=0)
    return (c_full, h_full)
